# revision 19
# baseline (speedup 1.0000x reference)
"""Talking-heads attention kernel for Trainium2, 8-core batch-parallel.

Problem (per batch element b, one NeuronCore each):
  q = x @ Wq; k,v = split(x @ Wkv)
  dots[h,i,j] = (q_h . k_h) * DH**-0.5
  dots' = einsum('hij,hg->gij', dots, mix_pre)
  attn  = softmax(dots', axis=j)
  attn' = einsum('hij,hg->gij', attn, mix_post)
  o_g   = attn'_g @ v_g ;  out = concat(o) @ Wo + bo

Device strategy (per core):
  - All matmul operands bf16 (PE runs 4x faster than fp32), fp32 PSUM accum.
  - x transposed on-chip (PE transpose) -> xT[c,i]; projections give
    qT[d,i], kT[d,j] and natural v[j,d].
  - QK runs transposed: scoresT[j, (h,i)] per j-tile (j on partitions).
  - Head mixes run on the PE via a transpose sandwich: a PE transpose of
    [j128, (12h x 8i)] chunks puts (h,i8) on partitions; mix_pre is a
    matmul with a host-built block-diagonal weight W1[(h,i1),(g,i2)] =
    mix_pre[h,g]*delta(i1,i2).
  - exp on ACT engine with accum_out giving row sums for free; softmax
    normalization folds into mix_post's stationary weights (W2 scaled
    per-partition by reciprocal sums on DVE).
  - mix_post uses swapped operands (lhsT = exp'd scores chunk, rhs = W2n)
    so its output lands back in [j, (g,i)] layout - undoing the transpose
    for free; AV and the output projection then run natively.
"""

import os
import time

import numpy as np
import ml_dtypes

import concourse.bass as bass
import concourse.tile as tile
from concourse import bacc
from concourse import mybir
from concourse.masks import make_identity

B, N, DIM, H, DH = 8, 1024, 768, 12, 64
INNER = H * DH  # 768
NCC = DIM // 128  # 6 chunks of the contraction/feature dims
NJT = N // 128  # 8 j-tiles
IB = 128  # i-block (query rows per block)
NBLK = N // IB  # 8
ISUB = 8  # i positions packed with the 12 heads: K = 12*8 = 96
NGRP = IB // ISUB  # 16 i8-groups per block
KP = H * ISUB  # 96 partitions used in mix land

BF16 = mybir.dt.bfloat16
F32 = mybir.dt.float32

_cache = {}
KSTAGE = int(os.environ.get("KSTAGE", "9"))
KALT = int(os.environ.get("KALT", "1"))
KBLKS = int(os.environ.get("KBLKS", str(NBLK)))
KLOOP = int(os.environ.get("KLOOP", "1"))


def _build(kloop=None):
    global KLOOP
    if kloop is not None:
        KLOOP = kloop
    nc = bacc.Bacc("TRN2", target_bir_lowering=False, debug=False)

    d_x = nc.dram_tensor("x", [N, DIM], BF16, kind="ExternalInput").ap()
    d_wq = nc.dram_tensor("wq", [DIM, INNER], BF16, kind="ExternalInput").ap()
    d_wk = nc.dram_tensor("wk", [DIM, INNER], BF16, kind="ExternalInput").ap()
    d_wv = nc.dram_tensor("wv", [DIM, INNER], BF16, kind="ExternalInput").ap()
    d_wo = nc.dram_tensor("wo", [INNER, DIM], BF16, kind="ExternalInput").ap()
    d_w1 = nc.dram_tensor("w1", [KP, KP], BF16, kind="ExternalInput").ap()
    d_w2 = nc.dram_tensor("w2", [KP, KP], F32, kind="ExternalInput").ap()
    d_bo = nc.dram_tensor("bo", [1, DIM], F32, kind="ExternalInput").ap()
    d_out = nc.dram_tensor("out", [N, DIM], F32, kind="ExternalOutput").ap()

    with tile.TileContext(nc) as tc:
        _body(tc, d_out, d_x, d_wq, d_wk, d_wv, d_wo, d_w1, d_w2, d_bo)
    nc.compile()
    return nc


def _body(tc, d_out, d_x, d_wq, d_wk, d_wv, d_wo, d_w1, d_w2, d_bo):
    nc = tc.nc
    from contextlib import ExitStack

    ctx = ExitStack()
    with ctx:
        consts = ctx.enter_context(tc.tile_pool(name="consts", bufs=1))
        sbig = ctx.enter_context(tc.tile_pool(name="sbig", bufs=1))
        stream = ctx.enter_context(tc.tile_pool(name="stream", bufs=4))
        small = ctx.enter_context(tc.tile_pool(name="small", bufs=6))
        # PSUM: 8 banks. misc 2x1 (QK/proj/AV/outproj/mix2 share the "mm"
        # slot - mix2 runs in a phase where the others are idle) + t1 2x1
        # + m1 2x2 (double-buffered so exp(g) overlaps mix1(g+1)) = 8.
        ps_misc = ctx.enter_context(tc.tile_pool(name="ps_misc", bufs=2, space="PSUM"))
        ps_t1 = ctx.enter_context(tc.tile_pool(name="ps_t1", bufs=2, space="PSUM"))
        ps_m1 = ctx.enter_context(tc.tile_pool(name="ps_m1", bufs=2, space="PSUM"))

        def psum_mm():
            return ps_misc.tile([128, 512], F32, name="mm", tag="mm")

        def psum_qk():
            return ps_misc.tile([128, 512], F32, name="qk", tag="mm")

        # ---- constants ----
        ident = consts.tile([128, 128], BF16, name="ident", tag="ident")
        make_identity(nc, ident)

        wq = [consts.tile([128, INNER], BF16, name=f"wq{i}", tag=f"wq{i}") for i in range(NCC)]
        wk = [consts.tile([128, INNER], BF16, name=f"wk{i}", tag=f"wk{i}") for i in range(NCC)]
        wv = [consts.tile([128, INNER], BF16, name=f"wv{i}", tag=f"wv{i}") for i in range(NCC)]
        wo = [consts.tile([128, DIM], BF16, name=f"wo{i}", tag=f"wo{i}") for i in range(NCC)]
        for cc in range(NCC):
            nc.sync.dma_start(wq[cc][:], d_wq[cc * 128:(cc + 1) * 128, :])
            nc.sync.dma_start(wk[cc][:], d_wk[cc * 128:(cc + 1) * 128, :])
            nc.sync.dma_start(wv[cc][:], d_wv[cc * 128:(cc + 1) * 128, :])
            nc.sync.dma_start(wo[cc][:], d_wo[cc * 128:(cc + 1) * 128, :])
        w1 = consts.tile([KP, KP], BF16, name="w1", tag="w1")
        nc.sync.dma_start(w1[:], d_w1[:, :])
        w2 = consts.tile([KP, KP], F32, name="w2", tag="w2")
        nc.sync.dma_start(w2[:], d_w2[:, :])
        bo_b = consts.tile([128, DIM], F32, name="bo", tag="bo")
        nc.gpsimd.dma_start(
            out=bo_b[:],
            in_=bass.AP(tensor=d_bo.tensor, offset=d_bo.offset,
                        ap=[[0, 128], [1, DIM]]),
        )

        # ---- x load + transpose -> xT[c, i] (6 tiles [128, 1024]) ----
        xload = ctx.enter_context(tc.tile_pool(name="xload", bufs=8))
        loop_ctx = tc.For_i(0, KLOOP, 1) if KLOOP > 1 else None
        if loop_ctx is not None:
            loop_ctx.__enter__()
        xT = [sbig.tile([128, N], BF16, name=f"xT{i}", tag=f"xT{i}") for i in range(NCC)]
        if True:
            for it in range(8):
                x_t = xload.tile([128, DIM], BF16, name="x_t", tag="x_t")
                nc.sync.dma_start(x_t[:], d_x[it * 128:(it + 1) * 128, :])
                for cc in range(NCC):
                    pt = ps_t1.tile([128, 128], BF16, name="ps_t1x", tag="ps_t1")
                    nc.tensor.transpose(pt[:], x_t[:, cc * 128:(cc + 1) * 128],
                                        ident[:])
                    eng = nc.vector if (cc % 2 == 0) else nc.scalar
                    if cc % 2 == 0:
                        eng.tensor_copy(xT[cc][:, it * 128:(it + 1) * 128], pt[:])
                    else:
                        eng.copy(xT[cc][:, it * 128:(it + 1) * 128], pt[:])

        # ---- projections ----
        qT = [sbig.tile([128, N], BF16, name=f"qT{i}", tag=f"qT{i}") for i in range(NCC)]
        kTz = [[sbig.tile([128, N], BF16, name=f"kTz{p}{i}", tag=f"kTz{p}{i}")
                for i in range(NCC)] for p in range(2)]
        for i in range(NCC):
            nc.gpsimd.memset(kTz[0][i][64:128, :], 0.0)
            nc.gpsimd.memset(kTz[1][i][0:64, :], 0.0)
        v = [sbig.tile([128, INNER], BF16, name=f"v{i}", tag=f"v{i}") for i in range(NJT)]
        for which, w in (("q", wq), ("k", wk)):
            for dc in range(NCC):
                for ih in range(2):
                    pp = psum_mm()
                    for cc in range(NCC):
                        nc.tensor.matmul(
                            pp[:],
                            lhsT=w[cc][:, dc * 128:(dc + 1) * 128],
                            rhs=xT[cc][:, ih * 512:(ih + 1) * 512],
                            start=(cc == 0), stop=(cc == NCC - 1),
                        )
                    sl = slice(ih * 512, (ih + 1) * 512)
                    if which == "q":
                        if dc % 2 == 0:
                            nc.vector.tensor_copy(qT[dc][:, sl], pp[:])
                        else:
                            nc.scalar.copy(qT[dc][:, sl], pp[:])
                    else:
                        nc.vector.tensor_copy(kTz[0][dc][0:64, sl], pp[0:64, :])
                        nc.scalar.copy(kTz[1][dc][64:128, sl], pp[64:128, :])
        for jt in range(NJT):
            for d0, dn in ((0, 512), (512, 256)):
                pp = psum_mm()
                for cc in range(NCC):
                    nc.tensor.matmul(
                        pp[:, :dn],
                        lhsT=xT[cc][:, jt * 128:(jt + 1) * 128],
                        rhs=wv[cc][:, d0:d0 + dn],
                        start=(cc == 0), stop=(cc == NCC - 1),
                    )
                if jt % 2 == 0:
                    nc.vector.tensor_copy(v[jt][:, d0:d0 + dn], pp[:, :dn])
                else:
                    nc.scalar.copy(v[jt][:, d0:d0 + dn], pp[:, :dn])

        if KSTAGE <= 1:
            dbg = stream.tile([128, DIM], F32, name="dbg", tag="out_sb")
            for it in range(8):
                nc.vector.tensor_copy(dbg[:], qT[it % NCC][:, :DIM])
                nc.sync.dma_start(d_out[it * 128:(it + 1) * 128, :], dbg[:])
            return

        # ---- main attention blocks ----
        s_all = [sbig.tile([128, H * IB], BF16, name=f"s{jt}", tag=f"s{jt}") for jt in range(NJT)]
        a_all = sbig.tile([128, NJT * H * IB], BF16, name="a_all", tag="a_all")
        oT = [sbig.tile([128, N], BF16, name=f"oT{i}", tag=f"oT{i}") for i in range(NCC)]

        for bi in range(KBLKS):
            # -- QK: scoresT[j, (h,i)] --
            for jt in range(NJT):
                for hs in range(3):  # 4 heads per psum bank
                    pq = psum_qk()
                    for hh in range(4):
                        h = hs * 4 + hh
                        dc, p = h // 2, h % 2
                        nc.tensor.matmul(
                            pq[:, hh * IB:(hh + 1) * IB],
                            lhsT=kTz[p][dc][:, jt * 128:(jt + 1) * 128],
                            rhs=qT[dc][:, bi * IB:(bi + 1) * IB],
                            start=True, stop=True,
                        )
                    sdst = s_all[jt][:].rearrange(
                        "p (g h i) -> p g h i", g=NGRP, h=H)[
                        :, :, hs * 4:(hs + 1) * 4, :]
                    ssrc = pq[:].rearrange("p (h g i) -> p g h i", h=4, g=NGRP)
                    if KALT == 0 or (jt + hs) % 2 == 0:
                        nc.vector.tensor_copy(sdst, ssrc)
                    else:
                        nc.scalar.copy(sdst, ssrc)

            if KSTAGE <= 2:
                dbg2 = stream.tile([128, DIM], F32, name="dbg2", tag="out_sb")
                nc.vector.tensor_copy(dbg2[:], s_all[bi][:, :DIM])
                nc.sync.dma_start(d_out[bi * IB:(bi + 1) * IB, :], dbg2[:])
                continue

            # -- per i8-group: transpose -> mix1 -> exp -> mix2(swap) --
            for g in range(NGRP):
                ts_sb = stream.tile([KP, N], BF16, name="ts", tag="ts")
                for jh in range(2):
                    pt = ps_t1.tile([KP, 512], BF16, name="ps_t1", tag="ps_t1")
                    for jr in range(4):
                        jt = jh * 4 + jr
                        nc.tensor.transpose(
                            pt[:, jr * 128:(jr + 1) * 128],
                            s_all[jt][:, g * KP:(g + 1) * KP], ident[:])
                    nc.vector.tensor_copy(
                        ts_sb[:, jh * 512:(jh + 1) * 512], pt[:])

                e_sb = stream.tile([KP, N], BF16, name="e", tag="e")
                ssum = small.tile([KP, 1], F32, name="ssum", tag="ssum")
                pm = ps_m1.tile([KP, 1024], F32, name="ps_m1", tag="ps_m1")
                for jh in range(2):
                    nc.tensor.matmul(
                        pm[:, jh * 512:(jh + 1) * 512],
                        lhsT=w1[:], rhs=ts_sb[:, jh * 512:(jh + 1) * 512],
                        start=True, stop=True,
                    )
                nc.scalar.activation(
                    e_sb[:], pm[:],
                    mybir.ActivationFunctionType.Exp,
                    accum_out=ssum[:],
                )
                rsum = small.tile([KP, 1], F32, name="rsum", tag="rsum")
                nc.vector.reciprocal(rsum[:], ssum[:])
                w2n = small.tile([KP, KP], BF16, name="w2n", tag="w2n")
                nc.gpsimd.tensor_scalar_mul(w2n[:], w2[:], rsum[:])

                # mix2 swapped: out[j, (g', i2)]; psum packs 4 j-tiles per bank
                for jh in range(2):
                    pm2 = ps_misc.tile([128, 4 * KP], F32, name="ps_m2", tag="mm")
                    for jr in range(4):
                        nc.tensor.matmul(
                            pm2[:, jr * KP:(jr + 1) * KP],
                            lhsT=e_sb[:, (jh * 4 + jr) * 128:(jh * 4 + jr + 1) * 128],
                            rhs=w2n[:],
                            start=True, stop=True,
                        )
                    # dest: a_all[p, jt*H*IB + gp*IB + (g*ISUB + i2)]
                    a_r = a_all[:].rearrange("p (t h i) -> p t h i", t=NJT, h=H)
                    dst = a_r[:, jh * 4:(jh + 1) * 4, :, g * ISUB:(g + 1) * ISUB]
                    src = pm2[:].rearrange("p (t h i) -> p t h i", t=4, h=H)
                    if (g * 2 + jh) % 8 >= 5:  # ~37% to ACT, rest DVE
                        nc.scalar.copy(dst, src)
                    else:
                        nc.vector.tensor_copy(dst, src)

            if KSTAGE <= 3:
                dbg3 = stream.tile([128, DIM], F32, name="dbg3", tag="out_sb")
                nc.vector.tensor_copy(
                    dbg3[:], a_all[:].rearrange("p (t x) -> p t x", t=NJT)[:, 0, :DIM])
                nc.sync.dma_start(d_out[bi * IB:(bi + 1) * IB, :], dbg3[:])
                continue

            # -- AV: oT[(g',d), i] --
            for gp in range(H):
                pav = ps_misc.tile([64, 128], F32, name="mm", tag="mm")
                a_r = a_all[:].rearrange("p (t h i) -> p t h i", t=NJT, h=H)
                for jt in range(NJT):
                    rhs = a_r[:, jt, gp, :]
                    nc.tensor.matmul(
                        pav[:],
                        lhsT=v[jt][:, gp * 64:(gp + 1) * 64],
                        rhs=rhs,
                        start=(jt == 0), stop=(jt == NJT - 1),
                    )
                dc, dp = gp // 2, (gp % 2) * 64
                if gp % 2 == 0:
                    nc.vector.tensor_copy(
                        oT[dc][dp:dp + 64, bi * IB:(bi + 1) * IB], pav[:])
                else:
                    nc.scalar.copy(
                        oT[dc][dp:dp + 64, bi * IB:(bi + 1) * IB], pav[:])

            # -- output projection + bias --
            out_sb = stream.tile([128, DIM], F32, name="out_sb", tag="out_sb")
            for e0, en in ((0, 512), (512, 256)):
                po = psum_mm()
                for gc in range(NCC):
                    nc.tensor.matmul(
                        po[:, :en],
                        lhsT=oT[gc][:, bi * IB:(bi + 1) * IB],
                        rhs=wo[gc][:, e0:e0 + en],
                        start=(gc == 0), stop=(gc == NCC - 1),
                    )
                nc.vector.tensor_add(out_sb[:, e0:e0 + en], po[:, :en],
                                     bo_b[:, e0:e0 + en])
            nc.sync.dma_start(d_out[bi * IB:(bi + 1) * IB, :], out_sb[:])
        if loop_ctx is not None:
            loop_ctx.__exit__(None, None, None)


def _prep_inputs(x, Wq, Wkv, mix_pre, mix_post, Wo, bo):
    """Host-side prep: dtype/layout only + tiny 12x12-scale weight builds."""
    bf16 = ml_dtypes.bfloat16
    scale = DH ** -0.5
    wq = (np.asarray(Wq, np.float32) * scale).astype(bf16)
    wk = np.ascontiguousarray(np.asarray(Wkv, np.float32)[:, :INNER]).astype(bf16)
    wv = np.ascontiguousarray(np.asarray(Wkv, np.float32)[:, INNER:]).astype(bf16)
    wo = np.asarray(Wo, np.float32).astype(bf16)
    m1 = np.asarray(mix_pre, np.float32)
    m2 = np.asarray(mix_post, np.float32)
    eye = np.eye(ISUB, dtype=np.float32)
    # W1[(h,i1),(g,i2)] = mix_pre[h,g] * delta(i1,i2)
    w1 = np.einsum("hg,ab->hagb", m1, eye).reshape(KP, KP).astype(bf16)
    # W2[(g,i1),(g2,i2)] = mix_post[g,g2] * delta(i1,i2)
    w2 = np.ascontiguousarray(
        np.einsum("hg,ab->hagb", m2, eye).reshape(KP, KP).astype(np.float32))
    bo_ = np.ascontiguousarray(np.asarray(bo, np.float32).reshape(1, DIM))
    in_maps = []
    for b in range(B):
        in_maps.append({
            "x": np.asarray(x[b], np.float32).astype(bf16),
            "wq": wq, "wk": wk, "wv": wv, "wo": wo,
            "w1": w1, "w2": w2, "bo": bo_,
        })
    return in_maps


def _get_nc(kloop=None):
    key = ("nc", kloop if kloop is not None else KLOOP)
    if key not in _cache:
        _cache[key] = _build(kloop)
    return _cache[key]


def _get_runner(nc):
    """Build (once) a jitted 8-core shard_map runner for `nc`.

    Returns (fn, in_names, out_avals). Cached so repeat kernel() calls
    reuse the compiled executable and skip re-lowering.
    """
    key = ("runner", id(nc))
    if key in _cache:
        return _cache[key]
    import jax
    from concourse import bass2jax, mybir as mb
    from jax.sharding import Mesh, PartitionSpec
    from jax.experimental.shard_map import shard_map

    bass2jax.install_neuronx_cc_hook()
    partition_name = nc.partition_id_tensor.name if nc.partition_id_tensor else None
    in_names, out_names, out_avals = [], [], []
    for alloc in nc.m.functions[0].allocations:
        if not isinstance(alloc, mb.MemoryLocationSet):
            continue
        name = alloc.memorylocations[0].name
        if alloc.kind == "ExternalInput":
            if name != partition_name:
                in_names.append(name)
        elif alloc.kind == "ExternalOutput":
            out_names.append(name)
            out_avals.append(jax.core.ShapedArray(
                tuple(alloc.tensor_shape), mb.dt.np(alloc.dtype)))
    n_params = len(in_names)
    all_in_names = list(in_names) + out_names
    if partition_name is not None:
        all_in_names = all_in_names + [partition_name]

    def _body(*args):
        operands = list(args)
        if partition_name is not None:
            operands.append(bass2jax.partition_id_tensor())
        outs = bass2jax._bass_exec_p.bind(
            *operands,
            out_avals=tuple(out_avals),
            in_names=tuple(all_in_names),
            out_names=tuple(out_names),
            lowering_input_output_aliases=(),
            sim_require_finite=True,
            sim_require_nnan=True,
            nc=nc,
        )
        return tuple(outs)

    devices = jax.devices()[:B]
    mesh = Mesh(np.asarray(devices), ("core",))
    specs = (PartitionSpec("core"),) * (n_params + len(out_names))
    out_specs = (PartitionSpec("core"),) * len(out_names)
    fn = jax.jit(shard_map(_body, mesh=mesh, in_specs=specs,
                           out_specs=out_specs, check_rep=False),
                 keep_unused=True)
    _cache[key] = (fn, in_names, out_avals)
    return _cache[key]


def _run_spmd(nc, in_maps):
    """Run `nc` on B cores (one in_map per core) via the cached runner."""
    import jax
    fn, in_names, out_avals = _get_runner(nc)
    n_cores = len(in_maps)
    concat_in = [np.concatenate([np.asarray(in_maps[c][nm])
                                 for c in range(n_cores)], axis=0)
                 for nm in in_names]
    concat_zero = [np.zeros((n_cores * a.shape[0], *a.shape[1:]), a.dtype)
                   for a in out_avals]
    args = [jax.device_put(a) for a in concat_in + concat_zero]
    out = fn(*args)
    jax.block_until_ready(out)
    outs_np = np.asarray(out[0]).reshape(n_cores, *out_avals[0].shape)
    return outs_np


def kernel(x, Wq, Wkv, mix_pre, mix_post, Wo, bo):
    nc = _get_nc(kloop=1)
    in_maps = _prep_inputs(x, Wq, Wkv, mix_pre, mix_post, Wo, bo)
    out = _run_spmd(nc, in_maps)
    return out.astype(np.float32)



# revision 21
# speedup vs baseline: 1.3418x; 1.3418x over previous
"""Talking-heads attention kernel for Trainium2, 8-core batch-parallel.

Problem (per batch element b, one NeuronCore each):
  q = x @ Wq; k,v = split(x @ Wkv)
  dots[h,i,j] = (q_h . k_h) * DH**-0.5
  dots' = einsum('hij,hg->gij', dots, mix_pre)
  attn  = softmax(dots', axis=j)
  attn' = einsum('hij,hg->gij', attn, mix_post)
  o_g   = attn'_g @ v_g ;  out = concat(o) @ Wo + bo

Device strategy (per core):
  - All matmul operands bf16 (PE runs 4x faster than fp32), fp32 PSUM accum.
  - x transposed on-chip (PE transpose) -> xT[c,i]; projections give
    qT[d,i], kT[d,j] and natural v[j,d].
  - QK runs transposed: scoresT[j, (h,i)] per j-tile (j on partitions).
  - Head mixes run on the PE via a transpose sandwich: a PE transpose of
    [j128, (12h x 8i)] chunks puts (h,i8) on partitions; mix_pre is a
    matmul with a host-built block-diagonal weight W1[(h,i1),(g,i2)] =
    mix_pre[h,g]*delta(i1,i2).
  - exp on ACT engine with accum_out giving row sums for free; softmax
    normalization folds into mix_post's stationary weights (W2 scaled
    per-partition by reciprocal sums on DVE).
  - mix_post uses swapped operands (lhsT = exp'd scores chunk, rhs = W2n)
    so its output lands back in [j, (g,i)] layout - undoing the transpose
    for free; AV and the output projection then run natively.
"""

import os
import time

import numpy as np
import ml_dtypes

import concourse.bass as bass
import concourse.tile as tile
from concourse import bacc
from concourse import mybir
from concourse.masks import make_identity

B, N, DIM, H, DH = 8, 1024, 768, 12, 64
INNER = H * DH  # 768
NCC = DIM // 128  # 6 chunks of the contraction/feature dims
NJT = N // 128  # 8 j-tiles
IB = 128  # i-block (query rows per block)
NBLK = N // IB  # 8
ISUB = 8  # i positions packed with the 12 heads: K = 12*8 = 96
NGRP = IB // ISUB  # 16 i8-groups per block
KP = H * ISUB  # 96 partitions used in mix land

BF16 = mybir.dt.bfloat16
F32 = mybir.dt.float32

_cache = {}
KSTAGE = int(os.environ.get("KSTAGE", "9"))
KALT = int(os.environ.get("KALT", "1"))
KBLKS = int(os.environ.get("KBLKS", str(NBLK)))
KLOOP = int(os.environ.get("KLOOP", "1"))


def _build(kloop=None):
    global KLOOP
    if kloop is not None:
        KLOOP = kloop
    nc = bacc.Bacc("TRN2", target_bir_lowering=False, debug=False)

    d_x = nc.dram_tensor("x", [N, DIM], BF16, kind="ExternalInput").ap()
    d_wq = nc.dram_tensor("wq", [DIM, INNER], BF16, kind="ExternalInput").ap()
    d_wk = nc.dram_tensor("wk", [DIM, INNER], BF16, kind="ExternalInput").ap()
    d_wv = nc.dram_tensor("wv", [DIM, INNER], BF16, kind="ExternalInput").ap()
    d_wo = nc.dram_tensor("wo", [INNER, DIM], BF16, kind="ExternalInput").ap()
    d_w1 = nc.dram_tensor("w1", [KP, KP], BF16, kind="ExternalInput").ap()
    d_w2 = nc.dram_tensor("w2", [KP, KP], F32, kind="ExternalInput").ap()
    d_bo = nc.dram_tensor("bo", [1, DIM], F32, kind="ExternalInput").ap()
    d_out = nc.dram_tensor("out", [N, DIM], F32, kind="ExternalOutput").ap()

    with tile.TileContext(nc) as tc:
        _body(tc, d_out, d_x, d_wq, d_wk, d_wv, d_wo, d_w1, d_w2, d_bo)
    nc.compile()
    return nc


def _body(tc, d_out, d_x, d_wq, d_wk, d_wv, d_wo, d_w1, d_w2, d_bo):
    nc = tc.nc
    from contextlib import ExitStack

    ctx = ExitStack()
    with ctx:
        consts = ctx.enter_context(tc.tile_pool(name="consts", bufs=1))
        sbig = ctx.enter_context(tc.tile_pool(name="sbig", bufs=1))
        stream = ctx.enter_context(tc.tile_pool(name="stream", bufs=4))
        small = ctx.enter_context(tc.tile_pool(name="small", bufs=6))
        # PSUM: 8 banks. misc 2x1 (QK/proj/AV/outproj/mix2 share the "mm"
        # slot - mix2 runs in a phase where the others are idle) + t1 2x1
        # + m1 2x2 (double-buffered so exp(g) overlaps mix1(g+1)) = 8.
        ps_misc = ctx.enter_context(tc.tile_pool(name="ps_misc", bufs=2, space="PSUM"))
        ps_t1 = ctx.enter_context(tc.tile_pool(name="ps_t1", bufs=2, space="PSUM"))
        ps_m1 = ctx.enter_context(tc.tile_pool(name="ps_m1", bufs=2, space="PSUM"))

        def psum_mm():
            return ps_misc.tile([128, 512], F32, name="mm", tag="mm")

        def psum_qk():
            return ps_misc.tile([128, 512], F32, name="qk", tag="mm")

        # ---- constants ----
        ident = consts.tile([128, 128], BF16, name="ident", tag="ident")
        make_identity(nc, ident)

        wq = [consts.tile([128, INNER], BF16, name=f"wq{i}", tag=f"wq{i}") for i in range(NCC)]
        wk = [consts.tile([128, INNER], BF16, name=f"wk{i}", tag=f"wk{i}") for i in range(NCC)]
        wv = [consts.tile([128, INNER], BF16, name=f"wv{i}", tag=f"wv{i}") for i in range(NCC)]
        wo = [consts.tile([128, DIM], BF16, name=f"wo{i}", tag=f"wo{i}") for i in range(NCC)]
        for cc in range(NCC):
            nc.sync.dma_start(wq[cc][:], d_wq[cc * 128:(cc + 1) * 128, :])
            nc.sync.dma_start(wk[cc][:], d_wk[cc * 128:(cc + 1) * 128, :])
            nc.sync.dma_start(wv[cc][:], d_wv[cc * 128:(cc + 1) * 128, :])
            nc.sync.dma_start(wo[cc][:], d_wo[cc * 128:(cc + 1) * 128, :])
        w1 = consts.tile([KP, KP], BF16, name="w1", tag="w1")
        nc.sync.dma_start(w1[:], d_w1[:, :])
        w2 = consts.tile([KP, KP], F32, name="w2", tag="w2")
        nc.sync.dma_start(w2[:], d_w2[:, :])
        bo_b = consts.tile([128, DIM], F32, name="bo", tag="bo")
        nc.gpsimd.dma_start(
            out=bo_b[:],
            in_=bass.AP(tensor=d_bo.tensor, offset=d_bo.offset,
                        ap=[[0, 128], [1, DIM]]),
        )

        # ---- x load + transpose -> xT[c, i] (6 tiles [128, 1024]) ----
        xload = ctx.enter_context(tc.tile_pool(name="xload", bufs=8))
        loop_ctx = tc.For_i(0, KLOOP, 1) if KLOOP > 1 else None
        if loop_ctx is not None:
            loop_ctx.__enter__()
        xT = [sbig.tile([128, N], BF16, name=f"xT{i}", tag=f"xT{i}") for i in range(NCC)]
        if True:
            for it in range(8):
                x_t = xload.tile([128, DIM], BF16, name="x_t", tag="x_t")
                nc.sync.dma_start(x_t[:], d_x[it * 128:(it + 1) * 128, :])
                for cc in range(NCC):
                    pt = ps_t1.tile([128, 128], BF16, name="ps_t1x", tag="ps_t1")
                    nc.tensor.transpose(pt[:], x_t[:, cc * 128:(cc + 1) * 128],
                                        ident[:])
                    eng = nc.vector if (cc % 2 == 0) else nc.scalar
                    if cc % 2 == 0:
                        eng.tensor_copy(xT[cc][:, it * 128:(it + 1) * 128], pt[:])
                    else:
                        eng.copy(xT[cc][:, it * 128:(it + 1) * 128], pt[:])

        # ---- projections ----
        qT = [sbig.tile([128, N], BF16, name=f"qT{i}", tag=f"qT{i}") for i in range(NCC)]
        kTz = [[sbig.tile([128, N], BF16, name=f"kTz{p}{i}", tag=f"kTz{p}{i}")
                for i in range(NCC)] for p in range(2)]
        for i in range(NCC):
            nc.vector.memset(kTz[0][i][64:128, :], 0.0)
            nc.vector.memset(kTz[1][i][0:64, :], 0.0)
        v = [sbig.tile([128, INNER], BF16, name=f"v{i}", tag=f"v{i}") for i in range(NJT)]
        for which, w in (("q", wq), ("k", wk)):
            for dc in range(NCC):
                for ih in range(2):
                    pp = psum_mm()
                    for cc in range(NCC):
                        nc.tensor.matmul(
                            pp[:],
                            lhsT=w[cc][:, dc * 128:(dc + 1) * 128],
                            rhs=xT[cc][:, ih * 512:(ih + 1) * 512],
                            start=(cc == 0), stop=(cc == NCC - 1),
                        )
                    sl = slice(ih * 512, (ih + 1) * 512)
                    if which == "q":
                        if dc % 2 == 0:
                            nc.vector.tensor_copy(qT[dc][:, sl], pp[:])
                        else:
                            nc.scalar.copy(qT[dc][:, sl], pp[:])
                    else:
                        nc.vector.tensor_copy(kTz[0][dc][0:64, sl], pp[0:64, :])
                        nc.scalar.copy(kTz[1][dc][64:128, sl], pp[64:128, :])
        for jt in range(NJT):
            for d0, dn in ((0, 512), (512, 256)):
                pp = psum_mm()
                for cc in range(NCC):
                    nc.tensor.matmul(
                        pp[:, :dn],
                        lhsT=xT[cc][:, jt * 128:(jt + 1) * 128],
                        rhs=wv[cc][:, d0:d0 + dn],
                        start=(cc == 0), stop=(cc == NCC - 1),
                    )
                if jt % 2 == 0:
                    nc.vector.tensor_copy(v[jt][:, d0:d0 + dn], pp[:, :dn])
                else:
                    nc.scalar.copy(v[jt][:, d0:d0 + dn], pp[:, :dn])

        if KSTAGE <= 1:
            dbg = stream.tile([128, DIM], F32, name="dbg", tag="out_sb")
            for it in range(8):
                nc.vector.tensor_copy(dbg[:], qT[it % NCC][:, :DIM])
                nc.sync.dma_start(d_out[it * 128:(it + 1) * 128, :], dbg[:])
            return

        # ---- main attention blocks ----
        s_all = [sbig.tile([128, H * IB], BF16, name=f"s{jt}", tag=f"s{jt}") for jt in range(NJT)]
        a_all = sbig.tile([128, NJT * H * IB], BF16, name="a_all", tag="a_all")
        oT = [sbig.tile([128, N], BF16, name=f"oT{i}", tag=f"oT{i}") for i in range(NCC)]

        for bi in range(KBLKS):
            # -- QK: scoresT[j, (h,i)] --
            for jt in range(NJT):
                for hs in range(3):  # 4 heads per psum bank
                    pq = psum_qk()
                    for hh in range(4):
                        h = hs * 4 + hh
                        dc, p = h // 2, h % 2
                        nc.tensor.matmul(
                            pq[:, hh * IB:(hh + 1) * IB],
                            lhsT=kTz[p][dc][:, jt * 128:(jt + 1) * 128],
                            rhs=qT[dc][:, bi * IB:(bi + 1) * IB],
                            start=True, stop=True,
                        )
                    sdst = s_all[jt][:].rearrange(
                        "p (g h i) -> p g h i", g=NGRP, h=H)[
                        :, :, hs * 4:(hs + 1) * 4, :]
                    ssrc = pq[:].rearrange("p (h g i) -> p g h i", h=4, g=NGRP)
                    if KALT == 0 or (jt + hs) % 2 == 0:
                        nc.vector.tensor_copy(sdst, ssrc)
                    else:
                        nc.scalar.copy(sdst, ssrc)

            if KSTAGE <= 2:
                dbg2 = stream.tile([128, DIM], F32, name="dbg2", tag="out_sb")
                nc.vector.tensor_copy(dbg2[:], s_all[bi][:, :DIM])
                nc.sync.dma_start(d_out[bi * IB:(bi + 1) * IB, :], dbg2[:])
                continue

            # -- per i8-group: transpose -> mix1 -> exp -> mix2(swap) --
            for g in range(NGRP):
                ts_sb = stream.tile([KP, N], BF16, name="ts", tag="ts")
                for jh in range(2):
                    pt = ps_t1.tile([KP, 512], BF16, name="ps_t1", tag="ps_t1")
                    for jr in range(4):
                        jt = jh * 4 + jr
                        nc.tensor.transpose(
                            pt[:, jr * 128:(jr + 1) * 128],
                            s_all[jt][:, g * KP:(g + 1) * KP], ident[:])
                    nc.vector.tensor_copy(
                        ts_sb[:, jh * 512:(jh + 1) * 512], pt[:])

                e_sb = stream.tile([KP, N], BF16, name="e", tag="e")
                ssum = small.tile([KP, 1], F32, name="ssum", tag="ssum")
                pm = ps_m1.tile([KP, 1024], F32, name="ps_m1", tag="ps_m1")
                for jh in range(2):
                    nc.tensor.matmul(
                        pm[:, jh * 512:(jh + 1) * 512],
                        lhsT=w1[:], rhs=ts_sb[:, jh * 512:(jh + 1) * 512],
                        start=True, stop=True,
                    )
                nc.scalar.activation(
                    e_sb[:], pm[:],
                    mybir.ActivationFunctionType.Exp,
                    accum_out=ssum[:],
                )
                rsum = small.tile([KP, 1], F32, name="rsum", tag="rsum")
                nc.vector.reciprocal(rsum[:], ssum[:])
                w2n = small.tile([KP, KP], BF16, name="w2n", tag="w2n")
                nc.vector.tensor_scalar_mul(w2n[:], w2[:], rsum[:])

                # mix2 swapped: out[j, (g', i2)]; psum packs 4 j-tiles per bank
                for jh in range(2):
                    pm2 = ps_misc.tile([128, 4 * KP], F32, name="ps_m2", tag="mm")
                    for jr in range(4):
                        nc.tensor.matmul(
                            pm2[:, jr * KP:(jr + 1) * KP],
                            lhsT=e_sb[:, (jh * 4 + jr) * 128:(jh * 4 + jr + 1) * 128],
                            rhs=w2n[:],
                            start=True, stop=True,
                        )
                    # dest: a_all[p, jt*H*IB + gp*IB + (g*ISUB + i2)]
                    a_r = a_all[:].rearrange("p (t h i) -> p t h i", t=NJT, h=H)
                    dst = a_r[:, jh * 4:(jh + 1) * 4, :, g * ISUB:(g + 1) * ISUB]
                    src = pm2[:].rearrange("p (t h i) -> p t h i", t=4, h=H)
                    if (g * 2 + jh) % 8 >= 5:  # ~37% to ACT, rest DVE
                        nc.scalar.copy(dst, src)
                    else:
                        nc.vector.tensor_copy(dst, src)

            if KSTAGE <= 3:
                dbg3 = stream.tile([128, DIM], F32, name="dbg3", tag="out_sb")
                nc.vector.tensor_copy(
                    dbg3[:], a_all[:].rearrange("p (t x) -> p t x", t=NJT)[:, 0, :DIM])
                nc.sync.dma_start(d_out[bi * IB:(bi + 1) * IB, :], dbg3[:])
                continue

            # -- AV: oT[(g',d), i] --
            for gp in range(H):
                pav = ps_misc.tile([64, 128], F32, name="mm", tag="mm")
                a_r = a_all[:].rearrange("p (t h i) -> p t h i", t=NJT, h=H)
                for jt in range(NJT):
                    rhs = a_r[:, jt, gp, :]
                    nc.tensor.matmul(
                        pav[:],
                        lhsT=v[jt][:, gp * 64:(gp + 1) * 64],
                        rhs=rhs,
                        start=(jt == 0), stop=(jt == NJT - 1),
                    )
                dc, dp = gp // 2, (gp % 2) * 64
                if gp % 2 == 0:
                    nc.vector.tensor_copy(
                        oT[dc][dp:dp + 64, bi * IB:(bi + 1) * IB], pav[:])
                else:
                    nc.scalar.copy(
                        oT[dc][dp:dp + 64, bi * IB:(bi + 1) * IB], pav[:])

            # -- output projection + bias --
            out_sb = stream.tile([128, DIM], F32, name="out_sb", tag="out_sb")
            for e0, en in ((0, 512), (512, 256)):
                po = psum_mm()
                for gc in range(NCC):
                    nc.tensor.matmul(
                        po[:, :en],
                        lhsT=oT[gc][:, bi * IB:(bi + 1) * IB],
                        rhs=wo[gc][:, e0:e0 + en],
                        start=(gc == 0), stop=(gc == NCC - 1),
                    )
                nc.vector.tensor_add(out_sb[:, e0:e0 + en], po[:, :en],
                                     bo_b[:, e0:e0 + en])
            nc.sync.dma_start(d_out[bi * IB:(bi + 1) * IB, :], out_sb[:])
        if loop_ctx is not None:
            loop_ctx.__exit__(None, None, None)


def _prep_inputs(x, Wq, Wkv, mix_pre, mix_post, Wo, bo):
    """Host-side prep: dtype/layout only + tiny 12x12-scale weight builds."""
    bf16 = ml_dtypes.bfloat16
    scale = DH ** -0.5
    wq = (np.asarray(Wq, np.float32) * scale).astype(bf16)
    wk = np.ascontiguousarray(np.asarray(Wkv, np.float32)[:, :INNER]).astype(bf16)
    wv = np.ascontiguousarray(np.asarray(Wkv, np.float32)[:, INNER:]).astype(bf16)
    wo = np.asarray(Wo, np.float32).astype(bf16)
    m1 = np.asarray(mix_pre, np.float32)
    m2 = np.asarray(mix_post, np.float32)
    eye = np.eye(ISUB, dtype=np.float32)
    # W1[(h,i1),(g,i2)] = mix_pre[h,g] * delta(i1,i2)
    w1 = np.einsum("hg,ab->hagb", m1, eye).reshape(KP, KP).astype(bf16)
    # W2[(g,i1),(g2,i2)] = mix_post[g,g2] * delta(i1,i2)
    w2 = np.ascontiguousarray(
        np.einsum("hg,ab->hagb", m2, eye).reshape(KP, KP).astype(np.float32))
    bo_ = np.ascontiguousarray(np.asarray(bo, np.float32).reshape(1, DIM))
    in_maps = []
    for b in range(B):
        in_maps.append({
            "x": np.asarray(x[b], np.float32).astype(bf16),
            "wq": wq, "wk": wk, "wv": wv, "wo": wo,
            "w1": w1, "w2": w2, "bo": bo_,
        })
    return in_maps


def _get_nc(kloop=None):
    key = ("nc", kloop if kloop is not None else KLOOP)
    if key not in _cache:
        _cache[key] = _build(kloop)
    return _cache[key]


def _get_runner(nc):
    """Build (once) a jitted 8-core shard_map runner for `nc`.

    Returns (fn, in_names, out_avals). Cached so repeat kernel() calls
    reuse the compiled executable and skip re-lowering.
    """
    key = ("runner", id(nc))
    if key in _cache:
        return _cache[key]
    import jax
    from concourse import bass2jax, mybir as mb
    from jax.sharding import Mesh, PartitionSpec
    from jax.experimental.shard_map import shard_map

    bass2jax.install_neuronx_cc_hook()
    partition_name = nc.partition_id_tensor.name if nc.partition_id_tensor else None
    in_names, out_names, out_avals = [], [], []
    for alloc in nc.m.functions[0].allocations:
        if not isinstance(alloc, mb.MemoryLocationSet):
            continue
        name = alloc.memorylocations[0].name
        if alloc.kind == "ExternalInput":
            if name != partition_name:
                in_names.append(name)
        elif alloc.kind == "ExternalOutput":
            out_names.append(name)
            out_avals.append(jax.core.ShapedArray(
                tuple(alloc.tensor_shape), mb.dt.np(alloc.dtype)))
    n_params = len(in_names)
    all_in_names = list(in_names) + out_names
    if partition_name is not None:
        all_in_names = all_in_names + [partition_name]

    def _body(*args):
        operands = list(args)
        if partition_name is not None:
            operands.append(bass2jax.partition_id_tensor())
        outs = bass2jax._bass_exec_p.bind(
            *operands,
            out_avals=tuple(out_avals),
            in_names=tuple(all_in_names),
            out_names=tuple(out_names),
            lowering_input_output_aliases=(),
            sim_require_finite=True,
            sim_require_nnan=True,
            nc=nc,
        )
        return tuple(outs)

    devices = jax.devices()[:B]
    mesh = Mesh(np.asarray(devices), ("core",))
    specs = (PartitionSpec("core"),) * (n_params + len(out_names))
    out_specs = (PartitionSpec("core"),) * len(out_names)
    fn = jax.jit(shard_map(_body, mesh=mesh, in_specs=specs,
                           out_specs=out_specs, check_rep=False),
                 keep_unused=True)
    _cache[key] = (fn, in_names, out_avals)
    return _cache[key]


def _run_spmd(nc, in_maps):
    """Run `nc` on B cores (one in_map per core) via the cached runner."""
    import jax
    fn, in_names, out_avals = _get_runner(nc)
    n_cores = len(in_maps)
    concat_in = [np.concatenate([np.asarray(in_maps[c][nm])
                                 for c in range(n_cores)], axis=0)
                 for nm in in_names]
    concat_zero = [np.zeros((n_cores * a.shape[0], *a.shape[1:]), a.dtype)
                   for a in out_avals]
    args = [jax.device_put(a) for a in concat_in + concat_zero]
    out = fn(*args)
    jax.block_until_ready(out)
    outs_np = np.asarray(out[0]).reshape(n_cores, *out_avals[0].shape)
    return outs_np


def kernel(x, Wq, Wkv, mix_pre, mix_post, Wo, bo):
    nc = _get_nc(kloop=1)
    in_maps = _prep_inputs(x, Wq, Wkv, mix_pre, mix_post, Wo, bo)
    out = _run_spmd(nc, in_maps)
    return out.astype(np.float32)



# revision 33
# speedup vs baseline: 1.4896x; 1.1101x over previous
"""Talking-heads attention kernel for Trainium2, 8-core batch-parallel.

Problem (per batch element b, one NeuronCore each):
  q = x @ Wq; k,v = split(x @ Wkv)
  dots[h,i,j] = (q_h . k_h) * DH**-0.5
  dots' = einsum('hij,hg->gij', dots, mix_pre)
  attn  = softmax(dots', axis=j)
  attn' = einsum('hij,hg->gij', attn, mix_post)
  o_g   = attn'_g @ v_g ;  out = concat(o) @ Wo + bo

Device strategy (per core):
  - All matmul operands bf16 (PE runs 4x faster than fp32), fp32 PSUM accum.
  - PSUM (8 banks): misc pool 2x1 bank shared by QK/proj/AV/outproj/mix2
    (phase-disjoint users), transpose pool 2x1, mix1 pool 2x2 banks so
    exp(group g) overlaps mix1(g+1) on the PE.
  - PSUM->SBUF copy traffic split across DVE and ACT to balance engine
    busy time against ACT's exp work (~300us each in CoreSim).
  - NOTE: keep GPSIMD out of the inner loop - its per-op launch overhead
    on real HW (~us) is far above the CoreSim Pool-engine cost model;
    moving the 128 w2n scalings + 12 memsets there cost +160us/iter HW.
  - x transposed on-chip (PE transpose) -> xT[c,i]; projections give
    qT[d,i], kT[d,j] and natural v[j,d].
  - QK runs transposed: scoresT[j, (h,i)] per j-tile (j on partitions).
  - Head mixes run on the PE via a transpose sandwich: a PE transpose of
    [j128, (12h x 8i)] chunks puts (h,i8) on partitions; mix_pre is a
    matmul with a host-built block-diagonal weight W1[(h,i1),(g,i2)] =
    mix_pre[h,g]*delta(i1,i2).
  - exp on ACT engine with accum_out giving row sums for free; softmax
    normalization folds into mix_post's stationary weights (W2 scaled
    per-partition by reciprocal sums on DVE).
  - mix_post uses swapped operands (lhsT = exp'd scores chunk, rhs = W2n)
    so its output lands back in [j, (g,i)] layout - undoing the transpose
    for free; AV and the output projection then run natively.
"""

import os
import time

import numpy as np
import ml_dtypes

import concourse.bass as bass
import concourse.tile as tile
from concourse import bacc
from concourse import mybir
from concourse.masks import make_identity

B, N, DIM, H, DH = 8, 1024, 768, 12, 64
INNER = H * DH  # 768
NCC = DIM // 128  # 6 chunks of the contraction/feature dims
NJT = N // 128  # 8 j-tiles
IB = 128  # max i-block (query rows per block)
NBLK = N // IB  # 8
ISUB = 8  # legacy: i positions packed with the 12 heads
NGRP = IB // ISUB
KP = H * ISUB
# Blocks: 8x120 rows with ISUB=10 (12 groups, 120/128 mix partitions used)
# + one 64-row tail with ISUB=8 (8 groups, 96 partitions). Vs uniform
# ISUB=8 this cuts exp cols and ts-copy volume ~19% and mix-land PE ~15%.
BLOCKS = [(i * 120, 120, 10) for i in range(8)] + [(960, 64, 8)]
KP10, KP8 = H * 10, H * 8  # 120, 96

BF16 = mybir.dt.bfloat16
F32 = mybir.dt.float32

_cache = {}
KSTAGE = int(os.environ.get("KSTAGE", "9"))
KALT = int(os.environ.get("KALT", "1"))
KBLKS = int(os.environ.get("KBLKS", str(NBLK)))
KLOOP = int(os.environ.get("KLOOP", "1"))


def _build(kloop=None):
    global KLOOP
    if kloop is not None:
        KLOOP = kloop
    nc = bacc.Bacc("TRN2", target_bir_lowering=False, debug=False)

    d_x = nc.dram_tensor("x", [N, DIM], BF16, kind="ExternalInput").ap()
    d_wq = nc.dram_tensor("wq", [DIM, INNER], BF16, kind="ExternalInput").ap()
    d_wk = nc.dram_tensor("wk", [DIM, INNER], BF16, kind="ExternalInput").ap()
    d_wv = nc.dram_tensor("wv", [DIM, INNER], BF16, kind="ExternalInput").ap()
    d_wo = nc.dram_tensor("wo", [INNER, DIM], BF16, kind="ExternalInput").ap()
    d_w1a = nc.dram_tensor("w1a", [KP10, KP10], BF16, kind="ExternalInput").ap()
    d_w1b = nc.dram_tensor("w1b", [KP8, KP8], BF16, kind="ExternalInput").ap()
    d_w2a = nc.dram_tensor("w2a", [KP10, KP10], F32, kind="ExternalInput").ap()
    d_w2b = nc.dram_tensor("w2b", [KP8, KP8], F32, kind="ExternalInput").ap()
    d_bo = nc.dram_tensor("bo", [1, DIM], F32, kind="ExternalInput").ap()
    d_out = nc.dram_tensor("out", [N, DIM], F32, kind="ExternalOutput").ap()

    with tile.TileContext(nc) as tc:
        _body(tc, d_out, d_x, d_wq, d_wk, d_wv, d_wo,
              (d_w1a, d_w1b), (d_w2a, d_w2b), d_bo)
    nc.compile()
    return nc


def _body(tc, d_out, d_x, d_wq, d_wk, d_wv, d_wo, d_w1s, d_w2s, d_bo):
    nc = tc.nc
    from contextlib import ExitStack

    ctx = ExitStack()
    with ctx:
        consts = ctx.enter_context(tc.tile_pool(name="consts", bufs=1))
        sbig = ctx.enter_context(tc.tile_pool(name="sbig", bufs=1))
        stream = ctx.enter_context(tc.tile_pool(name="stream", bufs=4))
        small = ctx.enter_context(tc.tile_pool(name="small", bufs=6))
        # PSUM: 8 banks. misc 2x1 (QK/proj/AV/outproj/mix2 share the "mm"
        # slot - mix2 runs in a phase where the others are idle) + t1 2x1
        # + m1 2x2 (double-buffered so exp(g) overlaps mix1(g+1)) = 8.
        ps_misc = ctx.enter_context(tc.tile_pool(name="ps_misc", bufs=2, space="PSUM"))
        ps_t1 = ctx.enter_context(tc.tile_pool(name="ps_t1", bufs=2, space="PSUM"))
        ps_m1 = ctx.enter_context(tc.tile_pool(name="ps_m1", bufs=2, space="PSUM"))

        def psum_mm():
            return ps_misc.tile([128, 512], F32, name="mm", tag="mm")

        def psum_qk():
            return ps_misc.tile([128, 512], F32, name="qk", tag="mm")

        # ---- constants ----
        ident = consts.tile([128, 128], BF16, name="ident", tag="ident")
        make_identity(nc, ident)

        wq = [consts.tile([128, INNER], BF16, name=f"wq{i}", tag=f"wq{i}") for i in range(NCC)]
        wk = [consts.tile([128, INNER], BF16, name=f"wk{i}", tag=f"wk{i}") for i in range(NCC)]
        wv = [consts.tile([128, INNER], BF16, name=f"wv{i}", tag=f"wv{i}") for i in range(NCC)]
        wo = [consts.tile([128, DIM], BF16, name=f"wo{i}", tag=f"wo{i}") for i in range(NCC)]
        for cc in range(NCC):
            nc.sync.dma_start(wq[cc][:], d_wq[cc * 128:(cc + 1) * 128, :])
            nc.sync.dma_start(wk[cc][:], d_wk[cc * 128:(cc + 1) * 128, :])
            nc.sync.dma_start(wv[cc][:], d_wv[cc * 128:(cc + 1) * 128, :])
            nc.sync.dma_start(wo[cc][:], d_wo[cc * 128:(cc + 1) * 128, :])
        w1a = consts.tile([KP10, KP10], BF16, name="w1a", tag="w1a")
        nc.sync.dma_start(w1a[:], d_w1s[0][:, :])
        w1b = consts.tile([KP8, KP8], BF16, name="w1b", tag="w1b")
        nc.sync.dma_start(w1b[:], d_w1s[1][:, :])
        w2a = consts.tile([KP10, KP10], F32, name="w2a", tag="w2a")
        nc.sync.dma_start(w2a[:], d_w2s[0][:, :])
        w2b = consts.tile([KP8, KP8], F32, name="w2b", tag="w2b")
        nc.sync.dma_start(w2b[:], d_w2s[1][:, :])
        bo_b = consts.tile([128, DIM], F32, name="bo", tag="bo")
        nc.gpsimd.dma_start(
            out=bo_b[:],
            in_=bass.AP(tensor=d_bo.tensor, offset=d_bo.offset,
                        ap=[[0, 128], [1, DIM]]),
        )

        # kTz zero halves are invariant across loop iterations - zero them
        # once outside the loop; the k-proj only rewrites the data halves.
        kTz = [[sbig.tile([128, N], BF16, name=f"kTz{p}{i}", tag=f"kTz{p}{i}")
                for i in range(NCC)] for p in range(2)]
        for i in range(NCC):
            nc.vector.memset(kTz[0][i][64:128, :], 0.0)
            nc.vector.memset(kTz[1][i][0:64, :], 0.0)

        # ---- x load + transpose -> xT[c, i] (6 tiles [128, 1024]) ----
        xload = ctx.enter_context(tc.tile_pool(name="xload", bufs=8))
        loop_ctx = tc.For_i(0, KLOOP, 1) if KLOOP > 1 else None
        if loop_ctx is not None:
            loop_ctx.__enter__()
        xT = [sbig.tile([128, N], BF16, name=f"xT{i}", tag=f"xT{i}") for i in range(NCC)]
        if True:
            for it in range(8):
                x_t = xload.tile([128, DIM], BF16, name="x_t", tag="x_t")
                nc.sync.dma_start(x_t[:], d_x[it * 128:(it + 1) * 128, :])
                for cc in range(NCC):
                    pt = ps_t1.tile([128, 128], BF16, name="ps_t1x", tag="ps_t1")
                    nc.tensor.transpose(pt[:], x_t[:, cc * 128:(cc + 1) * 128],
                                        ident[:])
                    eng = nc.vector if (cc % 2 == 0) else nc.scalar
                    if cc % 2 == 0:
                        eng.tensor_copy(xT[cc][:, it * 128:(it + 1) * 128], pt[:])
                    else:
                        eng.copy(xT[cc][:, it * 128:(it + 1) * 128], pt[:])

        # ---- projections ----
        qT = [sbig.tile([128, N], BF16, name=f"qT{i}", tag=f"qT{i}") for i in range(NCC)]
        v = [sbig.tile([128, INNER], BF16, name=f"v{i}", tag=f"v{i}") for i in range(NJT)]
        for which, w in (("q", wq), ("k", wk)):
            for dc in range(NCC):
                for ih in range(2):
                    pp = psum_mm()
                    for cc in range(NCC):
                        nc.tensor.matmul(
                            pp[:],
                            lhsT=w[cc][:, dc * 128:(dc + 1) * 128],
                            rhs=xT[cc][:, ih * 512:(ih + 1) * 512],
                            start=(cc == 0), stop=(cc == NCC - 1),
                        )
                    sl = slice(ih * 512, (ih + 1) * 512)
                    if which == "q":
                        if dc % 2 == 0:
                            nc.vector.tensor_copy(qT[dc][:, sl], pp[:])
                        else:
                            nc.scalar.copy(qT[dc][:, sl], pp[:])
                    else:
                        nc.vector.tensor_copy(kTz[0][dc][0:64, sl], pp[0:64, :])
                        nc.scalar.copy(kTz[1][dc][64:128, sl], pp[64:128, :])
        for jt in range(NJT):
            for d0, dn in ((0, 512), (512, 256)):
                pp = psum_mm()
                for cc in range(NCC):
                    nc.tensor.matmul(
                        pp[:, :dn],
                        lhsT=xT[cc][:, jt * 128:(jt + 1) * 128],
                        rhs=wv[cc][:, d0:d0 + dn],
                        start=(cc == 0), stop=(cc == NCC - 1),
                    )
                if jt % 2 == 0:
                    nc.vector.tensor_copy(v[jt][:, d0:d0 + dn], pp[:, :dn])
                else:
                    nc.scalar.copy(v[jt][:, d0:d0 + dn], pp[:, :dn])

        if KSTAGE <= 1:
            dbg = stream.tile([128, DIM], F32, name="dbg", tag="out_sb")
            for it in range(8):
                nc.vector.tensor_copy(dbg[:], qT[it % NCC][:, :DIM])
                nc.sync.dma_start(d_out[it * 128:(it + 1) * 128, :], dbg[:])
            return

        # ---- main attention blocks ----
        s_all = [sbig.tile([128, H * 120], BF16, name=f"s{jt}", tag=f"s{jt}") for jt in range(NJT)]
        a_all = sbig.tile([128, NJT * H * IB], BF16, name="a_all", tag="a_all")
        oT = [sbig.tile([128, N], BF16, name=f"oT{i}", tag=f"oT{i}") for i in range(NCC)]

        def emit_qk(i0, ib, isub):
            # QK: scoresT[j, (h,i)]
            ngrp = ib // isub
            for jt in range(NJT):
                for hs in range(3):  # 4 heads per psum bank
                    pq = psum_qk()
                    for hh in range(4):
                        h = hs * 4 + hh
                        dc, p = h // 2, h % 2
                        nc.tensor.matmul(
                            pq[:, hh * ib:(hh + 1) * ib],
                            lhsT=kTz[p][dc][:, jt * 128:(jt + 1) * 128],
                            rhs=qT[dc][:, i0:i0 + ib],
                            start=True, stop=True,
                        )
                    sdst = s_all[jt][:, :H * ib].rearrange(
                        "p (g h i) -> p g h i", g=ngrp, h=H)[
                        :, :, hs * 4:(hs + 1) * 4, :]
                    ssrc = pq[:, :4 * ib].rearrange("p (h g i) -> p g h i",
                                                    h=4, g=ngrp)
                    if KALT == 0 or (jt + hs) % 2 == 0:
                        nc.vector.tensor_copy(sdst, ssrc)
                    else:
                        nc.scalar.copy(sdst, ssrc)

        nblocks = len(BLOCKS) if KBLKS >= NBLK else KBLKS
        emit_qk(*BLOCKS[0])
        for bidx, (i0, ib, isub) in enumerate(BLOCKS[:nblocks]):
            kp = H * isub
            ngrp = ib // isub
            w1t = w1a if isub == 10 else w1b
            w2t = w2a if isub == 10 else w2b

            if KSTAGE <= 2:
                dbg2 = stream.tile([128, DIM], F32, name="dbg2", tag="out_sb")
                nc.vector.tensor_copy(dbg2[:ib], s_all[0][:ib, :DIM])
                nc.sync.dma_start(d_out[i0:i0 + ib, :], dbg2[:ib])
                continue

            # -- per i-group: transpose -> mix1 -> exp -> mix2(swap) --
            for g in range(ngrp):
                ts_sb = stream.tile([KP10, N], BF16, name="ts", tag="ts")
                for jh in range(2):
                    pt = ps_t1.tile([KP10, 512], BF16, name="ps_t1", tag="ps_t1")
                    for jr in range(4):
                        jt = jh * 4 + jr
                        nc.tensor.transpose(
                            pt[:kp, jr * 128:(jr + 1) * 128],
                            s_all[jt][:, g * kp:(g + 1) * kp], ident[:])
                    nc.vector.tensor_copy(
                        ts_sb[:kp, jh * 512:(jh + 1) * 512], pt[:kp, :])

                e_sb = stream.tile([KP10, N], BF16, name="e", tag="e")
                ssum = small.tile([KP10, 1], F32, name="ssum", tag="ssum")
                pm = ps_m1.tile([KP10, 1024], F32, name="ps_m1", tag="ps_m1")
                for jh in range(2):
                    nc.tensor.matmul(
                        pm[:kp, jh * 512:(jh + 1) * 512],
                        lhsT=w1t[:], rhs=ts_sb[:kp, jh * 512:(jh + 1) * 512],
                        start=True, stop=True,
                    )
                nc.scalar.activation(
                    e_sb[:kp, :], pm[:kp, :],
                    mybir.ActivationFunctionType.Exp,
                    accum_out=ssum[:kp],
                )
                rsum = small.tile([KP10, 1], F32, name="rsum", tag="rsum")
                nc.vector.reciprocal(rsum[:kp], ssum[:kp])
                w2n = small.tile([KP10, KP10], BF16, name="w2n", tag="w2n")
                nc.vector.tensor_scalar_mul(w2n[:kp, :kp], w2t[:], rsum[:kp])

                # mix2 swapped: out[j, (g', i2)]; psum packs 4 j-tiles per bank
                for jh in range(2):
                    pm2 = ps_misc.tile([128, 4 * KP10], F32, name="ps_m2",
                                       tag="mm")
                    for jr in range(4):
                        nc.tensor.matmul(
                            pm2[:, jr * kp:(jr + 1) * kp],
                            lhsT=e_sb[:kp, (jh * 4 + jr) * 128:(jh * 4 + jr + 1) * 128],
                            rhs=w2n[:kp, :kp],
                            start=True, stop=True,
                        )
                    # dest: a_all[p, jt*H*IB + gp*IB + (g*isub + i2)]
                    a_r = a_all[:].rearrange("p (t h i) -> p t h i", t=NJT, h=H)
                    dst = a_r[:, jh * 4:(jh + 1) * 4, :, g * isub:(g + 1) * isub]
                    src = pm2[:, :4 * kp].rearrange("p (t h i) -> p t h i",
                                                    t=4, h=H)
                    if (g * 2 + jh) % 8 >= 5:  # ~37% to ACT, rest DVE
                        nc.scalar.copy(dst, src)
                    else:
                        nc.vector.tensor_copy(dst, src)

            if KSTAGE <= 3:
                dbg3 = stream.tile([128, DIM], F32, name="dbg3", tag="out_sb")
                nc.vector.tensor_copy(
                    dbg3[:ib], a_all[:ib].rearrange("p (t x) -> p t x", t=NJT)[:, 0, :DIM])
                nc.sync.dma_start(d_out[i0:i0 + ib, :], dbg3[:ib])
                continue

            # -- AV: oT[(g',d), i] --
            for gp in range(H):
                pav = ps_misc.tile([64, 128], F32, name="mm", tag="mm")
                a_r = a_all[:].rearrange("p (t h i) -> p t h i", t=NJT, h=H)
                for jt in range(NJT):
                    rhs = a_r[:, jt, gp, 0:ib]
                    nc.tensor.matmul(
                        pav[:, :ib],
                        lhsT=v[jt][:, gp * 64:(gp + 1) * 64],
                        rhs=rhs,
                        start=(jt == 0), stop=(jt == NJT - 1),
                    )
                dc, dp = gp // 2, (gp % 2) * 64
                if gp % 2 == 0:
                    nc.vector.tensor_copy(
                        oT[dc][dp:dp + 64, i0:i0 + ib], pav[:, :ib])
                else:
                    nc.scalar.copy(
                        oT[dc][dp:dp + 64, i0:i0 + ib], pav[:, :ib])

            # Software pipeline: emit the NEXT block's QK before this
            # block's output projection, so DVE/ACT fill the otherwise
            # idle outproj tail (PE-serial) with the next scores copies.
            if bidx + 1 < nblocks:
                emit_qk(*BLOCKS[bidx + 1])

            # -- output projection + bias --
            out_sb = stream.tile([128, DIM], F32, name="out_sb", tag="out_sb")
            for e0, en in ((0, 512), (512, 256)):
                po = psum_mm()
                for gc in range(NCC):
                    nc.tensor.matmul(
                        po[:ib, :en],
                        lhsT=oT[gc][:, i0:i0 + ib],
                        rhs=wo[gc][:, e0:e0 + en],
                        start=(gc == 0), stop=(gc == NCC - 1),
                    )
                nc.vector.tensor_add(out_sb[:ib, e0:e0 + en], po[:ib, :en],
                                     bo_b[:ib, e0:e0 + en])
            nc.sync.dma_start(d_out[i0:i0 + ib, :], out_sb[:ib, :])
        if loop_ctx is not None:
            loop_ctx.__exit__(None, None, None)


def _prep_inputs(x, Wq, Wkv, mix_pre, mix_post, Wo, bo):
    """Host-side prep: dtype/layout only + tiny 12x12-scale weight builds."""
    bf16 = ml_dtypes.bfloat16
    scale = DH ** -0.5
    wq = (np.asarray(Wq, np.float32) * scale).astype(bf16)
    wk = np.ascontiguousarray(np.asarray(Wkv, np.float32)[:, :INNER]).astype(bf16)
    wv = np.ascontiguousarray(np.asarray(Wkv, np.float32)[:, INNER:]).astype(bf16)
    wo = np.asarray(Wo, np.float32).astype(bf16)
    m1 = np.asarray(mix_pre, np.float32)
    m2 = np.asarray(mix_post, np.float32)

    def mk_w(m, sub, dtype):
        eye = np.eye(sub, dtype=np.float32)
        # W[(h,i1),(g,i2)] = m[h,g] * delta(i1,i2)
        return np.ascontiguousarray(
            np.einsum("hg,ab->hagb", m, eye).reshape(H * sub, H * sub)
            .astype(dtype))

    w1a, w1b = mk_w(m1, 10, bf16), mk_w(m1, 8, bf16)
    w2a, w2b = mk_w(m2, 10, np.float32), mk_w(m2, 8, np.float32)
    bo_ = np.ascontiguousarray(np.asarray(bo, np.float32).reshape(1, DIM))
    in_maps = []
    for b in range(B):
        in_maps.append({
            "x": np.asarray(x[b], np.float32).astype(bf16),
            "wq": wq, "wk": wk, "wv": wv, "wo": wo,
            "w1a": w1a, "w1b": w1b, "w2a": w2a, "w2b": w2b, "bo": bo_,
        })
    return in_maps


def _get_nc(kloop=None):
    key = ("nc", kloop if kloop is not None else KLOOP)
    if key not in _cache:
        _cache[key] = _build(kloop)
    return _cache[key]


def _get_runner(nc):
    """Build (once) a jitted 8-core shard_map runner for `nc`.

    Returns (fn, in_names, out_avals). Cached so repeat kernel() calls
    reuse the compiled executable and skip re-lowering.
    """
    key = ("runner", id(nc))
    if key in _cache:
        return _cache[key]
    import jax
    from concourse import bass2jax, mybir as mb
    from jax.sharding import Mesh, PartitionSpec
    from jax.experimental.shard_map import shard_map

    bass2jax.install_neuronx_cc_hook()
    partition_name = nc.partition_id_tensor.name if nc.partition_id_tensor else None
    in_names, out_names, out_avals = [], [], []
    for alloc in nc.m.functions[0].allocations:
        if not isinstance(alloc, mb.MemoryLocationSet):
            continue
        name = alloc.memorylocations[0].name
        if alloc.kind == "ExternalInput":
            if name != partition_name:
                in_names.append(name)
        elif alloc.kind == "ExternalOutput":
            out_names.append(name)
            out_avals.append(jax.core.ShapedArray(
                tuple(alloc.tensor_shape), mb.dt.np(alloc.dtype)))
    n_params = len(in_names)
    all_in_names = list(in_names) + out_names
    if partition_name is not None:
        all_in_names = all_in_names + [partition_name]

    def _body(*args):
        operands = list(args)
        if partition_name is not None:
            operands.append(bass2jax.partition_id_tensor())
        outs = bass2jax._bass_exec_p.bind(
            *operands,
            out_avals=tuple(out_avals),
            in_names=tuple(all_in_names),
            out_names=tuple(out_names),
            lowering_input_output_aliases=(),
            sim_require_finite=True,
            sim_require_nnan=True,
            nc=nc,
        )
        return tuple(outs)

    devices = jax.devices()[:B]
    mesh = Mesh(np.asarray(devices), ("core",))
    specs = (PartitionSpec("core"),) * (n_params + len(out_names))
    out_specs = (PartitionSpec("core"),) * len(out_names)
    fn = jax.jit(shard_map(_body, mesh=mesh, in_specs=specs,
                           out_specs=out_specs, check_rep=False),
                 keep_unused=True)
    _cache[key] = (fn, in_names, out_avals)
    return _cache[key]


def _run_spmd(nc, in_maps):
    """Run `nc` on B cores (one in_map per core) via the cached runner."""
    import jax
    fn, in_names, out_avals = _get_runner(nc)
    n_cores = len(in_maps)
    concat_in = [np.concatenate([np.asarray(in_maps[c][nm])
                                 for c in range(n_cores)], axis=0)
                 for nm in in_names]
    concat_zero = [np.zeros((n_cores * a.shape[0], *a.shape[1:]), a.dtype)
                   for a in out_avals]
    args = [jax.device_put(a) for a in concat_in + concat_zero]
    out = fn(*args)
    jax.block_until_ready(out)
    outs_np = np.asarray(out[0]).reshape(n_cores, *out_avals[0].shape)
    return outs_np


def kernel(x, Wq, Wkv, mix_pre, mix_post, Wo, bo):
    nc = _get_nc(kloop=1)
    in_maps = _prep_inputs(x, Wq, Wkv, mix_pre, mix_post, Wo, bo)
    out = _run_spmd(nc, in_maps)
    return out.astype(np.float32)



# revision 35
# speedup vs baseline: 1.4970x; 1.0050x over previous
"""Talking-heads attention kernel for Trainium2, 8-core batch-parallel.

Problem (per batch element b, one NeuronCore each):
  q = x @ Wq; k,v = split(x @ Wkv)
  dots[h,i,j] = (q_h . k_h) * DH**-0.5
  dots' = einsum('hij,hg->gij', dots, mix_pre)
  attn  = softmax(dots', axis=j)
  attn' = einsum('hij,hg->gij', attn, mix_post)
  o_g   = attn'_g @ v_g ;  out = concat(o) @ Wo + bo

Device strategy (per core):
  - All matmul operands bf16 (PE runs 4x faster than fp32), fp32 PSUM accum.
  - i-blocks: 8x120 rows with ISUB=10 (12 uniform (h,i10)=120-partition mix
    groups) + one 64-row tail with ISUB=8 (96 partitions). Vs uniform
    ISUB=8/96p this cuts exp columns, ts-copy volume, and mix-land PE
    cycles ~17% (25% -> 6% partition-lane waste).
  - PSUM (8 banks): misc pool 2x1 bank shared by QK/proj/AV/outproj/mix2
    (phase-disjoint users), transpose pool 2x1, mix1 pool 2x2 banks so
    exp(group g) overlaps mix1(g+1) on the PE.
  - Software pipeline: the next block's QK matmuls+copies are emitted
    before this block's output projection, filling the PE-serial tail
    where DVE/ACT would otherwise idle.
  - kTz zero halves are memset once outside the KLOOP hardware loop
    (loop iterations only rewrite the data halves).
  - PSUM->SBUF copy traffic split across DVE and ACT to balance engine
    busy time against ACT's exp work (~285us each in CoreSim).
  - NOTE: keep GPSIMD out of the inner loop - its per-op launch overhead
    on real HW (~us) is far above the CoreSim Pool-engine cost model;
    moving the 128 w2n scalings + 12 memsets there cost +160us/iter HW.
  - x transposed on-chip (PE transpose) -> xT[c,i]; projections give
    qT[d,i], kT[d,j] and natural v[j,d].
  - QK runs transposed: scoresT[j, (h,i)] per j-tile (j on partitions).
  - Head mixes run on the PE via a transpose sandwich: a PE transpose of
    [j128, (12h x 8i)] chunks puts (h,i8) on partitions; mix_pre is a
    matmul with a host-built block-diagonal weight W1[(h,i1),(g,i2)] =
    mix_pre[h,g]*delta(i1,i2).
  - exp on ACT engine with accum_out giving row sums for free; softmax
    normalization folds into mix_post's stationary weights (W2 scaled
    per-partition by reciprocal sums on DVE).
  - mix_post uses swapped operands (lhsT = exp'd scores chunk, rhs = W2n)
    so its output lands back in [j, (g,i)] layout - undoing the transpose
    for free; AV and the output projection then run natively.
"""

import os
import time

import numpy as np
import ml_dtypes

import concourse.bass as bass
import concourse.tile as tile
from concourse import bacc
from concourse import mybir
from concourse.masks import make_identity

B, N, DIM, H, DH = 8, 1024, 768, 12, 64
INNER = H * DH  # 768
NCC = DIM // 128  # 6 chunks of the contraction/feature dims
NJT = N // 128  # 8 j-tiles
IB = 128  # max i-block (query rows per block)
NBLK = N // IB  # 8
ISUB = 8  # legacy: i positions packed with the 12 heads
NGRP = IB // ISUB
KP = H * ISUB
# Blocks: 8x120 rows with ISUB=10 (12 groups, 120/128 mix partitions used)
# + one 64-row tail with ISUB=8 (8 groups, 96 partitions). Vs uniform
# ISUB=8 this cuts exp cols and ts-copy volume ~19% and mix-land PE ~15%.
BLOCKS = [(i * 120, 120, 10) for i in range(8)] + [(960, 64, 8)]
KP10, KP8 = H * 10, H * 8  # 120, 96

BF16 = mybir.dt.bfloat16
F32 = mybir.dt.float32

_cache = {}
KSTAGE = int(os.environ.get("KSTAGE", "9"))
KALT = int(os.environ.get("KALT", "1"))
KBLKS = int(os.environ.get("KBLKS", str(NBLK)))
KLOOP = int(os.environ.get("KLOOP", "1"))


def _build(kloop=None):
    global KLOOP
    if kloop is not None:
        KLOOP = kloop
    nc = bacc.Bacc("TRN2", target_bir_lowering=False, debug=False)

    d_x = nc.dram_tensor("x", [N, DIM], BF16, kind="ExternalInput").ap()
    d_wq = nc.dram_tensor("wq", [DIM, INNER], BF16, kind="ExternalInput").ap()
    d_wk = nc.dram_tensor("wk", [DIM, INNER], BF16, kind="ExternalInput").ap()
    d_wv = nc.dram_tensor("wv", [DIM, INNER], BF16, kind="ExternalInput").ap()
    d_wo = nc.dram_tensor("wo", [INNER, DIM], BF16, kind="ExternalInput").ap()
    d_w1a = nc.dram_tensor("w1a", [KP10, KP10], BF16, kind="ExternalInput").ap()
    d_w1b = nc.dram_tensor("w1b", [KP8, KP8], BF16, kind="ExternalInput").ap()
    d_w2a = nc.dram_tensor("w2a", [KP10, KP10], F32, kind="ExternalInput").ap()
    d_w2b = nc.dram_tensor("w2b", [KP8, KP8], F32, kind="ExternalInput").ap()
    d_bo = nc.dram_tensor("bo", [1, DIM], F32, kind="ExternalInput").ap()
    d_out = nc.dram_tensor("out", [N, DIM], F32, kind="ExternalOutput").ap()

    with tile.TileContext(nc) as tc:
        _body(tc, d_out, d_x, d_wq, d_wk, d_wv, d_wo,
              (d_w1a, d_w1b), (d_w2a, d_w2b), d_bo)
    nc.compile()
    return nc


def _body(tc, d_out, d_x, d_wq, d_wk, d_wv, d_wo, d_w1s, d_w2s, d_bo):
    nc = tc.nc
    from contextlib import ExitStack

    ctx = ExitStack()
    with ctx:
        consts = ctx.enter_context(tc.tile_pool(name="consts", bufs=1))
        sbig = ctx.enter_context(tc.tile_pool(name="sbig", bufs=1))
        stream = ctx.enter_context(tc.tile_pool(name="stream", bufs=4))
        small = ctx.enter_context(tc.tile_pool(name="small", bufs=6))
        # PSUM: 8 banks. misc 2x1 (QK/proj/AV/outproj/mix2 share the "mm"
        # slot - mix2 runs in a phase where the others are idle) + t1 2x1
        # + m1 2x2 (double-buffered so exp(g) overlaps mix1(g+1)) = 8.
        ps_misc = ctx.enter_context(tc.tile_pool(name="ps_misc", bufs=2, space="PSUM"))
        ps_t1 = ctx.enter_context(tc.tile_pool(name="ps_t1", bufs=2, space="PSUM"))
        ps_m1 = ctx.enter_context(tc.tile_pool(name="ps_m1", bufs=2, space="PSUM"))

        def psum_mm():
            return ps_misc.tile([128, 512], F32, name="mm", tag="mm")

        def psum_qk():
            return ps_misc.tile([128, 512], F32, name="qk", tag="mm")

        # ---- constants ----
        ident = consts.tile([128, 128], BF16, name="ident", tag="ident")
        make_identity(nc, ident)

        wq = [consts.tile([128, INNER], BF16, name=f"wq{i}", tag=f"wq{i}") for i in range(NCC)]
        wk = [consts.tile([128, INNER], BF16, name=f"wk{i}", tag=f"wk{i}") for i in range(NCC)]
        wv = [consts.tile([128, INNER], BF16, name=f"wv{i}", tag=f"wv{i}") for i in range(NCC)]
        wo = [consts.tile([128, DIM], BF16, name=f"wo{i}", tag=f"wo{i}") for i in range(NCC)]
        for cc in range(NCC):
            nc.sync.dma_start(wq[cc][:], d_wq[cc * 128:(cc + 1) * 128, :])
            nc.sync.dma_start(wk[cc][:], d_wk[cc * 128:(cc + 1) * 128, :])
            nc.sync.dma_start(wv[cc][:], d_wv[cc * 128:(cc + 1) * 128, :])
            nc.sync.dma_start(wo[cc][:], d_wo[cc * 128:(cc + 1) * 128, :])
        w1a = consts.tile([KP10, KP10], BF16, name="w1a", tag="w1a")
        nc.sync.dma_start(w1a[:], d_w1s[0][:, :])
        w1b = consts.tile([KP8, KP8], BF16, name="w1b", tag="w1b")
        nc.sync.dma_start(w1b[:], d_w1s[1][:, :])
        w2a = consts.tile([KP10, KP10], F32, name="w2a", tag="w2a")
        nc.sync.dma_start(w2a[:], d_w2s[0][:, :])
        w2b = consts.tile([KP8, KP8], F32, name="w2b", tag="w2b")
        nc.sync.dma_start(w2b[:], d_w2s[1][:, :])
        bo_b = consts.tile([128, DIM], F32, name="bo", tag="bo")
        nc.gpsimd.dma_start(
            out=bo_b[:],
            in_=bass.AP(tensor=d_bo.tensor, offset=d_bo.offset,
                        ap=[[0, 128], [1, DIM]]),
        )

        # kTz zero halves are invariant across loop iterations - zero them
        # once outside the loop; the k-proj only rewrites the data halves.
        kTz = [[sbig.tile([128, N], BF16, name=f"kTz{p}{i}", tag=f"kTz{p}{i}")
                for i in range(NCC)] for p in range(2)]
        for i in range(NCC):
            nc.vector.memset(kTz[0][i][64:128, :], 0.0)
            nc.vector.memset(kTz[1][i][0:64, :], 0.0)

        # ---- x load + transpose -> xT[c, i] (6 tiles [128, 1024]) ----
        xload = ctx.enter_context(tc.tile_pool(name="xload", bufs=8))
        loop_ctx = tc.For_i(0, KLOOP, 1) if KLOOP > 1 else None
        if loop_ctx is not None:
            loop_ctx.__enter__()
        xT = [sbig.tile([128, N], BF16, name=f"xT{i}", tag=f"xT{i}") for i in range(NCC)]
        if True:
            for it in range(8):
                x_t = xload.tile([128, DIM], BF16, name="x_t", tag="x_t")
                nc.sync.dma_start(x_t[:], d_x[it * 128:(it + 1) * 128, :])
                for cc in range(NCC):
                    pt = ps_t1.tile([128, 128], BF16, name="ps_t1x", tag="ps_t1")
                    nc.tensor.transpose(pt[:], x_t[:, cc * 128:(cc + 1) * 128],
                                        ident[:])
                    eng = nc.vector if (cc % 2 == 0) else nc.scalar
                    if cc % 2 == 0:
                        eng.tensor_copy(xT[cc][:, it * 128:(it + 1) * 128], pt[:])
                    else:
                        eng.copy(xT[cc][:, it * 128:(it + 1) * 128], pt[:])

        # ---- projections ----
        qT = [sbig.tile([128, N], BF16, name=f"qT{i}", tag=f"qT{i}") for i in range(NCC)]
        v = [sbig.tile([128, INNER], BF16, name=f"v{i}", tag=f"v{i}") for i in range(NJT)]
        for which, w in (("q", wq), ("k", wk)):
            for dc in range(NCC):
                for ih in range(2):
                    pp = psum_mm()
                    for cc in range(NCC):
                        nc.tensor.matmul(
                            pp[:],
                            lhsT=w[cc][:, dc * 128:(dc + 1) * 128],
                            rhs=xT[cc][:, ih * 512:(ih + 1) * 512],
                            start=(cc == 0), stop=(cc == NCC - 1),
                        )
                    sl = slice(ih * 512, (ih + 1) * 512)
                    if which == "q":
                        if dc % 2 == 0:
                            nc.vector.tensor_copy(qT[dc][:, sl], pp[:])
                        else:
                            nc.scalar.copy(qT[dc][:, sl], pp[:])
                    else:
                        nc.vector.tensor_copy(kTz[0][dc][0:64, sl], pp[0:64, :])
                        nc.scalar.copy(kTz[1][dc][64:128, sl], pp[64:128, :])
        for jt in range(NJT):
            for d0, dn in ((0, 512), (512, 256)):
                pp = psum_mm()
                for cc in range(NCC):
                    nc.tensor.matmul(
                        pp[:, :dn],
                        lhsT=xT[cc][:, jt * 128:(jt + 1) * 128],
                        rhs=wv[cc][:, d0:d0 + dn],
                        start=(cc == 0), stop=(cc == NCC - 1),
                    )
                if jt % 2 == 0:
                    nc.vector.tensor_copy(v[jt][:, d0:d0 + dn], pp[:, :dn])
                else:
                    nc.scalar.copy(v[jt][:, d0:d0 + dn], pp[:, :dn])

        if KSTAGE <= 1:
            dbg = stream.tile([128, DIM], F32, name="dbg", tag="out_sb")
            for it in range(8):
                nc.vector.tensor_copy(dbg[:], qT[it % NCC][:, :DIM])
                nc.sync.dma_start(d_out[it * 128:(it + 1) * 128, :], dbg[:])
            return

        # ---- main attention blocks ----
        s_all = [sbig.tile([128, H * 120], BF16, name=f"s{jt}", tag=f"s{jt}") for jt in range(NJT)]
        a_all = sbig.tile([128, NJT * H * IB], BF16, name="a_all", tag="a_all")
        oT = [sbig.tile([128, N], BF16, name=f"oT{i}", tag=f"oT{i}") for i in range(NCC)]

        def emit_qk(i0, ib, isub):
            # QK: scoresT[j, (h,i)]
            ngrp = ib // isub
            for jt in range(NJT):
                for hs in range(3):  # 4 heads per psum bank
                    pq = psum_qk()
                    for hh in range(4):
                        h = hs * 4 + hh
                        dc, p = h // 2, h % 2
                        nc.tensor.matmul(
                            pq[:, hh * ib:(hh + 1) * ib],
                            lhsT=kTz[p][dc][:, jt * 128:(jt + 1) * 128],
                            rhs=qT[dc][:, i0:i0 + ib],
                            start=True, stop=True,
                        )
                    sdst = s_all[jt][:, :H * ib].rearrange(
                        "p (g h i) -> p g h i", g=ngrp, h=H)[
                        :, :, hs * 4:(hs + 1) * 4, :]
                    ssrc = pq[:, :4 * ib].rearrange("p (h g i) -> p g h i",
                                                    h=4, g=ngrp)
                    if KALT == 0 or (jt + hs) % 2 == 0:
                        nc.vector.tensor_copy(sdst, ssrc)
                    else:
                        nc.scalar.copy(sdst, ssrc)

        nblocks = len(BLOCKS) if KBLKS >= NBLK else KBLKS
        emit_qk(*BLOCKS[0])
        for bidx, (i0, ib, isub) in enumerate(BLOCKS[:nblocks]):
            kp = H * isub
            ngrp = ib // isub
            w1t = w1a if isub == 10 else w1b
            w2t = w2a if isub == 10 else w2b

            if KSTAGE <= 2:
                dbg2 = stream.tile([128, DIM], F32, name="dbg2", tag="out_sb")
                nc.vector.tensor_copy(dbg2[:ib], s_all[0][:ib, :DIM])
                nc.sync.dma_start(d_out[i0:i0 + ib, :], dbg2[:ib])
                continue

            # -- per i-group: transpose -> mix1 -> exp -> mix2(swap) --
            for g in range(ngrp):
                ts_sb = stream.tile([KP10, N], BF16, name="ts", tag="ts")
                for jh in range(2):
                    pt = ps_t1.tile([KP10, 512], BF16, name="ps_t1", tag="ps_t1")
                    for jr in range(4):
                        jt = jh * 4 + jr
                        nc.tensor.transpose(
                            pt[:kp, jr * 128:(jr + 1) * 128],
                            s_all[jt][:, g * kp:(g + 1) * kp], ident[:])
                    nc.vector.tensor_copy(
                        ts_sb[:kp, jh * 512:(jh + 1) * 512], pt[:kp, :])

                e_sb = stream.tile([KP10, N], BF16, name="e", tag="e")
                ssum = small.tile([KP10, 1], F32, name="ssum", tag="ssum")
                pm = ps_m1.tile([KP10, 1024], F32, name="ps_m1", tag="ps_m1")
                for jh in range(2):
                    nc.tensor.matmul(
                        pm[:kp, jh * 512:(jh + 1) * 512],
                        lhsT=w1t[:], rhs=ts_sb[:kp, jh * 512:(jh + 1) * 512],
                        start=True, stop=True,
                    )
                nc.scalar.activation(
                    e_sb[:kp, :], pm[:kp, :],
                    mybir.ActivationFunctionType.Exp,
                    accum_out=ssum[:kp],
                )
                rsum = small.tile([KP10, 1], F32, name="rsum", tag="rsum")
                nc.vector.reciprocal(rsum[:kp], ssum[:kp])
                w2n = small.tile([KP10, KP10], BF16, name="w2n", tag="w2n")
                nc.vector.tensor_scalar_mul(w2n[:kp, :kp], w2t[:], rsum[:kp])

                # mix2 swapped: out[j, (g', i2)]; psum packs 4 j-tiles per bank
                for jh in range(2):
                    pm2 = ps_misc.tile([128, 4 * KP10], F32, name="ps_m2",
                                       tag="mm")
                    for jr in range(4):
                        nc.tensor.matmul(
                            pm2[:, jr * kp:(jr + 1) * kp],
                            lhsT=e_sb[:kp, (jh * 4 + jr) * 128:(jh * 4 + jr + 1) * 128],
                            rhs=w2n[:kp, :kp],
                            start=True, stop=True,
                        )
                    # dest: a_all[p, jt*H*IB + gp*IB + (g*isub + i2)]
                    a_r = a_all[:].rearrange("p (t h i) -> p t h i", t=NJT, h=H)
                    dst = a_r[:, jh * 4:(jh + 1) * 4, :, g * isub:(g + 1) * isub]
                    src = pm2[:, :4 * kp].rearrange("p (t h i) -> p t h i",
                                                    t=4, h=H)
                    if (g * 2 + jh) % 2 == 1:  # 50/50 DVE/ACT
                        nc.scalar.copy(dst, src)
                    else:
                        nc.vector.tensor_copy(dst, src)

            if KSTAGE <= 3:
                dbg3 = stream.tile([128, DIM], F32, name="dbg3", tag="out_sb")
                nc.vector.tensor_copy(
                    dbg3[:ib], a_all[:ib].rearrange("p (t x) -> p t x", t=NJT)[:, 0, :DIM])
                nc.sync.dma_start(d_out[i0:i0 + ib, :], dbg3[:ib])
                continue

            # -- AV: oT[(g',d), i] --
            for gp in range(H):
                pav = ps_misc.tile([64, 128], F32, name="mm", tag="mm")
                a_r = a_all[:].rearrange("p (t h i) -> p t h i", t=NJT, h=H)
                for jt in range(NJT):
                    rhs = a_r[:, jt, gp, 0:ib]
                    nc.tensor.matmul(
                        pav[:, :ib],
                        lhsT=v[jt][:, gp * 64:(gp + 1) * 64],
                        rhs=rhs,
                        start=(jt == 0), stop=(jt == NJT - 1),
                    )
                dc, dp = gp // 2, (gp % 2) * 64
                if gp % 2 == 0:
                    nc.vector.tensor_copy(
                        oT[dc][dp:dp + 64, i0:i0 + ib], pav[:, :ib])
                else:
                    nc.scalar.copy(
                        oT[dc][dp:dp + 64, i0:i0 + ib], pav[:, :ib])

            # Software pipeline: emit the NEXT block's QK before this
            # block's output projection, so DVE/ACT fill the otherwise
            # idle outproj tail (PE-serial) with the next scores copies.
            if bidx + 1 < nblocks:
                emit_qk(*BLOCKS[bidx + 1])

            # -- output projection + bias --
            out_sb = stream.tile([128, DIM], F32, name="out_sb", tag="out_sb")
            for e0, en in ((0, 512), (512, 256)):
                po = psum_mm()
                for gc in range(NCC):
                    nc.tensor.matmul(
                        po[:ib, :en],
                        lhsT=oT[gc][:, i0:i0 + ib],
                        rhs=wo[gc][:, e0:e0 + en],
                        start=(gc == 0), stop=(gc == NCC - 1),
                    )
                nc.vector.tensor_add(out_sb[:ib, e0:e0 + en], po[:ib, :en],
                                     bo_b[:ib, e0:e0 + en])
            nc.sync.dma_start(d_out[i0:i0 + ib, :], out_sb[:ib, :])
        if loop_ctx is not None:
            loop_ctx.__exit__(None, None, None)


def _prep_inputs(x, Wq, Wkv, mix_pre, mix_post, Wo, bo):
    """Host-side prep: dtype/layout only + tiny 12x12-scale weight builds."""
    bf16 = ml_dtypes.bfloat16
    scale = DH ** -0.5
    wq = (np.asarray(Wq, np.float32) * scale).astype(bf16)
    wk = np.ascontiguousarray(np.asarray(Wkv, np.float32)[:, :INNER]).astype(bf16)
    wv = np.ascontiguousarray(np.asarray(Wkv, np.float32)[:, INNER:]).astype(bf16)
    wo = np.asarray(Wo, np.float32).astype(bf16)
    m1 = np.asarray(mix_pre, np.float32)
    m2 = np.asarray(mix_post, np.float32)

    def mk_w(m, sub, dtype):
        eye = np.eye(sub, dtype=np.float32)
        # W[(h,i1),(g,i2)] = m[h,g] * delta(i1,i2)
        return np.ascontiguousarray(
            np.einsum("hg,ab->hagb", m, eye).reshape(H * sub, H * sub)
            .astype(dtype))

    w1a, w1b = mk_w(m1, 10, bf16), mk_w(m1, 8, bf16)
    w2a, w2b = mk_w(m2, 10, np.float32), mk_w(m2, 8, np.float32)
    bo_ = np.ascontiguousarray(np.asarray(bo, np.float32).reshape(1, DIM))
    in_maps = []
    for b in range(B):
        in_maps.append({
            "x": np.asarray(x[b], np.float32).astype(bf16),
            "wq": wq, "wk": wk, "wv": wv, "wo": wo,
            "w1a": w1a, "w1b": w1b, "w2a": w2a, "w2b": w2b, "bo": bo_,
        })
    return in_maps


def _get_nc(kloop=None):
    key = ("nc", kloop if kloop is not None else KLOOP)
    if key not in _cache:
        _cache[key] = _build(kloop)
    return _cache[key]


def _get_runner(nc):
    """Build (once) a jitted 8-core shard_map runner for `nc`.

    Returns (fn, in_names, out_avals). Cached so repeat kernel() calls
    reuse the compiled executable and skip re-lowering.
    """
    key = ("runner", id(nc))
    if key in _cache:
        return _cache[key]
    import jax
    from concourse import bass2jax, mybir as mb
    from jax.sharding import Mesh, PartitionSpec
    from jax.experimental.shard_map import shard_map

    bass2jax.install_neuronx_cc_hook()
    partition_name = nc.partition_id_tensor.name if nc.partition_id_tensor else None
    in_names, out_names, out_avals = [], [], []
    for alloc in nc.m.functions[0].allocations:
        if not isinstance(alloc, mb.MemoryLocationSet):
            continue
        name = alloc.memorylocations[0].name
        if alloc.kind == "ExternalInput":
            if name != partition_name:
                in_names.append(name)
        elif alloc.kind == "ExternalOutput":
            out_names.append(name)
            out_avals.append(jax.core.ShapedArray(
                tuple(alloc.tensor_shape), mb.dt.np(alloc.dtype)))
    n_params = len(in_names)
    all_in_names = list(in_names) + out_names
    if partition_name is not None:
        all_in_names = all_in_names + [partition_name]

    def _body(*args):
        operands = list(args)
        if partition_name is not None:
            operands.append(bass2jax.partition_id_tensor())
        outs = bass2jax._bass_exec_p.bind(
            *operands,
            out_avals=tuple(out_avals),
            in_names=tuple(all_in_names),
            out_names=tuple(out_names),
            lowering_input_output_aliases=(),
            sim_require_finite=True,
            sim_require_nnan=True,
            nc=nc,
        )
        return tuple(outs)

    devices = jax.devices()[:B]
    mesh = Mesh(np.asarray(devices), ("core",))
    specs = (PartitionSpec("core"),) * (n_params + len(out_names))
    out_specs = (PartitionSpec("core"),) * len(out_names)
    fn = jax.jit(shard_map(_body, mesh=mesh, in_specs=specs,
                           out_specs=out_specs, check_rep=False),
                 keep_unused=True)
    _cache[key] = (fn, in_names, out_avals)
    return _cache[key]


def _run_spmd(nc, in_maps):
    """Run `nc` on B cores (one in_map per core) via the cached runner."""
    import jax
    fn, in_names, out_avals = _get_runner(nc)
    n_cores = len(in_maps)
    concat_in = [np.concatenate([np.asarray(in_maps[c][nm])
                                 for c in range(n_cores)], axis=0)
                 for nm in in_names]
    concat_zero = [np.zeros((n_cores * a.shape[0], *a.shape[1:]), a.dtype)
                   for a in out_avals]
    args = [jax.device_put(a) for a in concat_in + concat_zero]
    out = fn(*args)
    jax.block_until_ready(out)
    outs_np = np.asarray(out[0]).reshape(n_cores, *out_avals[0].shape)
    return outs_np


def kernel(x, Wq, Wkv, mix_pre, mix_post, Wo, bo):
    nc = _get_nc(kloop=1)
    in_maps = _prep_inputs(x, Wq, Wkv, mix_pre, mix_post, Wo, bo)
    out = _run_spmd(nc, in_maps)
    return out.astype(np.float32)

